# revision 40
# baseline (speedup 1.0000x reference)
"""LocallyHierarchicalNet Trainium2 kernel (fp16, weight-stationary,
coalesced weight stream, AllToAll batch-resharded tail).

Net: 10 locally-connected conv1d layers (kernel=stride=2, unshared weights
per position), B=128, C_in=3, H=256, D=1024, then mean + linear head.

Strategy (8 NeuronCores, SPMD):
  - Position-shard layers 0-6: core i owns output positions [i*64,(i+1)*64)
    of layer 0, narrowing to exactly 1 position at layer 6 with zero
    cross-core traffic (binary-tree locality).
  - One 64KB AllToAll reshards from position- to batch-sharding at the
    x6 boundary: core c sends batch slice [16j,16j+16) of its (already
    relu'd fp16) layer-6 output to core j, receiving all 8 x6 positions
    for its own 16-element batch slice. Layers 7-9 + head then run
    batch-sharded (free dim 16), and each core writes output rows
    [16c,16c+16); the host concatenates the 8 slices. The exchanged
    payload is the same either way, but costing the collective on 64KB
    instead of an AllGather's 512KB output saves ~11.5us, and exchanging
    x6 itself (not w7 partials) drops the partial-matmul + cast hop from
    the pre-collective critical path.
  - All weights/activations are fp16 on device (host pre-casts), halving
    the HBM weight stream (the bottleneck: ~271MB of once-used weights,
    ~17MB/core; fp8 was measured at 3.3% final error -- over the gate).
    PSUM accumulation stays f32, so only input rounding (~1e-3) is
    incurred.
  - Matmul layout: weight-stationary. Per output position and O-half:
    psum[o',b] += sum_{kk,ch} W[c',o'] . X[c', b]; lhsT = weight chunk
    [c'=128, o'=128], rhs = activation chunk [c'=128, B] (1 cyc/row fp16).
    Output lands directly as [O, B] = next layer's [C, B]: no transposes.
  - Fused relu+scale+fp16-cast psum->SBUF ops alternate between the
    Activation and DVE engines (GPSIMD cannot read PSUM). L1/L2 bulk and
    L3 positions are processed two per psum tile / relu op; L4 positions
    split per O-half across engines; the L5/L6 root chain uses whole-
    position relus on alternating pinned engines.
  - HBM layout: x0+w0 fused into one tensor; layer-1..6 weights fused into
    one position-major stream in layer order (pooled 2MB slabs for the
    L1/L2 bulk; small dependency-free DMAs into a persistent tile for the
    L3..L6 tail, per-position singles pacing the root chain). Layer-0
    pairs weave between L1 pairs to fill PE while slabs are in flight.
    Layer-7..9 weights + head beta ride in one transfer under the
    collective, held off the pre-collective DMA window by a
    tile_wait_until scheduling floor (the Tile scheduler otherwise hoists
    the dep-free 5us transfer in front of the latency-critical x6 store).
    A tile_wait_until-gated stream of free-dim-512 matmuls keeps the PE
    p-state ramped across the collective so tail matmuls run at full
    clock.

Timeline (cost-model sim): stream 2-48.7us (DMA-saturated, 360GB/s),
root chain + x6 store -> collective launch 54.2us, AllToAll 16.6us
(15us constant + 64KB/40GBps), batch-sharded tail + output 9.5us.
"""

import sys

sys.path.insert(0, "/opt/trn_rl_repo")

import numpy as np

N_CORES = 8
B = 128
BT = B // N_CORES  # per-core batch slice for the post-exchange tail layers
C_IN = 3
H = 256
OUT = 10

# per-core output positions per layer (layers 7-9 replicated on all cores)
NPOS = {1: 32, 2: 16, 3: 8, 4: 4, 5: 2, 6: 1, 7: 4, 8: 2, 9: 1}
# position order for the fused layer-1..6 weight stream: plain layer order.
# All L3 weights land ~5us before stream end (absorbing their relu latency),
# and per-position singles for L4/L5/L6 release the root chain step by step.
TAIL = [
    (3, 0), (3, 1), (3, 2), (3, 3), (3, 4), (3, 5), (3, 6), (3, 7),
    (4, 0), (4, 1), (4, 2), (4, 3), (5, 0), (5, 1), (6, 0),
]
POS_A = (
    [(1, p) for p in range(32)] + [(2, p) for p in range(16)] + TAIL
)
# post-exchange layers 7-9 read weights from the replicated wb stream
# slab boundaries: big pooled slabs for the L1/L2 bulk; the interleaved tree
# tail (g 48..62) lives in one persistent SBUF tile filled by small
# dependency-free DMAs so completion sems release compute position-by-position
SLABS_BULK = [(0, 8), (8, 16), (16, 24), (24, 32), (32, 40), (40, 48)]
SLABS_TAIL = [(48, 52), (52, 56), (56, 57), (57, 58), (58, 59), (59, 60),
              (60, 61), (61, 62), (62, 63)]

# scheduler-time floor (ms) for the replicated layer-7..9 weight transfer:
# late enough to keep it off the pre-collective DMA window, early enough to
# finish under the collective
WSB_WAIT_MS = 0.086
KEEPER_WAIT_MS = 0.085

_NC = None


def _build():
    import concourse.bacc as bacc
    import concourse.mybir as mybir
    import concourse.tile as tile

    dt = mybir.dt
    f16 = dt.float16
    f32 = dt.float32
    Relu = mybir.ActivationFunctionType.Relu
    Copy = mybir.ActivationFunctionType.Copy
    Mult = mybir.AluOpType.mult
    Max = mybir.AluOpType.max

    nc = bacc.Bacc(
        "TRN2", target_bir_lowering=False, debug=False, num_devices=N_CORES
    )

    xw0_d = nc.dram_tensor("xw0", [6, 64 * B + 64 * H], f16, kind="ExternalInput")
    wa_d = nc.dram_tensor("wa", [128, 63 * 1024], f16, kind="ExternalInput")
    wb_d = nc.dram_tensor("wb", [128, 7 * 1024 + 2 * OUT], f16, kind="ExternalInput")
    out_d = nc.dram_tensor("out", [BT, OUT], f32, kind="ExternalOutput")

    # relu engine round-robin counter
    rr = [0]

    with tile.TileContext(nc) as tc:
        with (
            tc.tile_pool(name="sb", bufs=1) as sb,
            tc.tile_pool(name="wp", bufs=3) as wp,
            tc.tile_pool(name="psp", bufs=6, space="PSUM") as psp,
            tc.tile_pool(name="php", bufs=1, space="PSUM") as php,
            tc.tile_pool(name="psk", bufs=1, space="PSUM") as psk,
            tc.tile_pool(name="dram", bufs=1, space="DRAM") as dp,
        ):

            # NOTE: GPSIMD/Pool cannot read PSUM (BIR verifier), so the
            # relu/scale/cast work alternates between Activation and DVE only
            def relu_one(e, dst, src, scale):
                if e == 0:
                    nc.scalar.activation(dst, src, Relu, scale=scale)
                else:
                    nc.vector.tensor_scalar(dst, src, scale, 0.0, Mult, Max)

            def relu_store(dst, src, scale):
                relu_one(rr[0] % 2, dst, src, scale)
                rr[0] += 1

            def relu_store2(dst, src, scale):
                # latency-critical: the two O-halves on two different engines
                # so downstream consumers start after half 0
                e = rr[0]
                rr[0] += 1
                relu_one(e % 2, dst[:, 0, :], src[:, 0, :], scale)
                relu_one((e + 1) % 2, dst[:, 1, :], src[:, 1, :], scale)

            xw0 = sb.tile([6, 64 * B + 64 * H], f16, tag="xw0", name="xw0_sb")
            nc.sync.dma_start(xw0[:], xw0_d[:])
            W0 = 64 * B  # offset of the w0 block within xw0

            # activation chain tiles: X[l] is the input of layer l (l=1..10);
            # X[7] is replaced by the exchanged X7g below. Layers 8+ run
            # batch-sharded: only this core's 16-element batch slice.
            X = {1: sb.tile([128, 64, 2, B], f16, tag="x1", name="X1")}
            for l in range(1, 10):
                bl = B if l < 7 else BT
                X[l + 1] = sb.tile(
                    [128, NPOS[l], 2, bl], f16, tag=f"x{l + 1}", name=f"X{l + 1}"
                )

            # ---- layer 0: K=6 (kk,c_in); lhsT = w0 [6, o-chunk], rhs = x [6, B]
            # output written directly in chain layout [o', oh, B]; two
            # positions share a psum tile and one relu op. L0 pairs are
            # interleaved into the layer-1 slab stream: they depend only on
            # xw0, so they fill PE while weight slabs are still in flight.
            s3 = 1.0 / (3.0**0.5)
            l0_next = [0]

            def l0_pair():
                pos = l0_next[0]
                l0_next[0] += 2
                pt = psp.tile([128, 2, 2, B], f32, tag="pt", name=f"p0_{pos}")
                for q in range(2):
                    for j in range(2):
                        o0 = W0 + (pos + q) * H + j * 128
                        nc.tensor.matmul(
                            pt[:, q, j, :],
                            xw0[:, o0 : o0 + 128],
                            xw0[:, (pos + q) * B : (pos + q + 1) * B],
                            start=True,
                            stop=True,
                        )
                relu_store(X[1][:, pos : pos + 2, :, :], pt[:], s3)

            # front-load enough L0 pairs to keep PE busy until slab 1 lands
            for _ in range(8):
                l0_pair()

            def mms(l, p, ws, off, pslice, X7g=None):
                """The 8 matmuls of one layer-l output position into pslice."""
                Xin = X7g if l == 7 else X[l]
                for oh in range(2):
                    for ci in range(4):
                        kk, ch = ci >> 1, ci & 1
                        o = off + ((kk * 2 + ch) * 2 + oh) * 128
                        nc.tensor.matmul(
                            pslice[:, oh, :],
                            ws[:, o : o + 128],
                            Xin[:, 2 * p + kk, ch, :],
                            start=(ci == 0),
                            stop=(ci == 3),
                        )

            def do_pair(l, p, ws, off):
                """Two consecutive positions, one psum tile, one relu op."""
                pt = psp.tile([128, 2, 2, B], f32, tag="pt", name=f"pt{l}_{p}")
                mms(l, p, ws, off, pt[:, 0, :, :])
                mms(l, p + 1, ws, off + 1024, pt[:, 1, :, :])
                relu_store(X[l + 1][:, p : p + 2, :, :], pt[:], 1.0 / 16.0)

            def do_pos(l, p, ws, off, X7g=None, split=False, eng=None):
                """One position; split=True spreads the relu halves over two
                engines (latency-critical tree tail); eng pins the engine."""
                bl = B if l < 7 else BT
                pt = psp.tile([128, 2, 2, bl], f32, tag="pt", name=f"pt{l}_{p}")
                mms(l, p, ws, off, pt[:, 0, :, :], X7g=X7g)
                if eng is not None:
                    relu_one(eng, X[l + 1][:, p, :, :], pt[:, 0, :, :], 1.0 / 16.0)
                else:
                    st = relu_store2 if split else relu_store
                    st(X[l + 1][:, p, :, :], pt[:, 0, :, :], 1.0 / 16.0)

            # ---- layers 1-2 bulk: pooled slab stream, positions in pairs,
            # remaining L0 pairs woven between L1 pairs
            for g0, g1 in SLABS_BULK:
                ws = wp.tile([128, (g1 - g0) * 1024], f16, tag="ws", name=f"wsa_{g0}")
                nc.sync.dma_start(ws[:], wa_d[:, g0 * 1024 : g1 * 1024])
                for g in range(g0, g1, 2):
                    l, p = POS_A[g]
                    if l == 1:
                        # keep L0 coverage ahead of this pair's inputs
                        while l0_next[0] < min(64, 2 * p + 12):
                            l0_pair()
                    do_pair(l, p, ws, (g - g0) * 1024)
            while l0_next[0] < 64:
                l0_pair()

            # ---- layer 3-6 tree tail: persistent tile, fine-grained DMAs
            T0 = 48
            wt = sb.tile([128, 15 * 1024], f16, tag="wt", name="wtail")
            for g0, g1 in SLABS_TAIL:
                nc.sync.dma_start(
                    wt[:, (g0 - T0) * 1024 : (g1 - T0) * 1024],
                    wa_d[:, g0 * 1024 : g1 * 1024],
                )
            # L3 (g 48-55) processes positions in pairs: one relu op per two
            # positions halves the fixed per-op engine overhead, relieving
            # ACT/DVE throughput pressure in the window where L4's
            # latency-critical relus also need those engines. Final
            # root-chain positions (g>=56) stay per-position: L4 splits
            # halves across engines, L5/L6 use whole-position relus on
            # alternating pinned engines.
            PIN = {59: 1, 60: 0, 61: 1}
            for g in range(48, 56, 2):
                l, p = POS_A[g]
                do_pair(l, p, wt, (g - T0) * 1024)
            for g in range(56, 63):
                l, p = POS_A[g]
                do_pos(l, p, wt, (g - T0) * 1024, split=(g not in PIN),
                       eng=PIN.get(g))

            # PE warm-keeper: dependency-free free-dim-512 matmuls in a
            # dedicated psum bank keep the PE p-state ramped across the
            # collective's idle gap (~20us) so the post-exchange layer
            # matmuls run at full clock the moment their data lands.
            wpt = psk.tile([128, 512], f32, tag="wpt", name="warm_pt")
            with tc.tile_wait_until(KEEPER_WAIT_MS):
                for _ in range(110):
                    nc.tensor.matmul(
                        wpt[:],
                        xw0[:, 0:128],
                        xw0[:, B : B + 512],
                        start=True,
                        stop=True,
                    )

            # ---- AllToAll the layer-6 outputs (x6, already relu'd fp16 in
            # X[7], so no extra cast/partial hop before the exchange): chunk
            # j of this core's x6 (batch slice [16j,16j+16)) goes to core j;
            # output chunk j is core j's x6 position for THIS core's batch
            # slice. 64KB each way.
            ag_in = dp.tile([N_CORES, 128, 2, BT], f16, name="ag_in")
            ag_out = dp.tile([N_CORES, 128, 2, BT], f16, name="ag_out")
            x6v = X[7].rearrange("p q oh (j b) -> p q oh j b", j=N_CORES)
            agi = ag_in.rearrange("j p oh b -> p oh j b")
            nc.sync.dma_start(agi[:, 0:1, :, :], x6v[:, 0, 0:1, :, :])
            nc.sync.dma_start(agi[:, 1:2, :, :], x6v[:, 0, 1:2, :, :])
            nc.gpsimd.collective_compute(
                "AllToAll",
                mybir.AluOpType.bypass,
                replica_groups=[list(range(N_CORES))],
                ins=[ag_in.opt()],
                outs=[ag_out.opt()],
            )

            # layers 7-9 weights + beta: single transfer, hidden under the
            # collective. tile_wait_until keeps the Tile scheduler from
            # hoisting this dep-free 5us transfer into the DMA window the
            # latency-critical ag store needs just before the collective.
            wsb = wp.tile([128, 7 * 1024 + 2 * OUT], f16, tag="ws", name="wsb")
            with tc.tile_wait_until(WSB_WAIT_MS):
                nc.sync.dma_start(wsb[:], wb_d[:])

            # exchanged x6 (all 8 positions) for this core's batch slice
            X7g = sb.tile([128, 8, 2, BT], f16, tag="x7g", name="X7g")
            nc.sync.dma_start(X7g[:], ag_out.rearrange("j p oh b -> p j oh b"))

            # ---- layers 7-9 for this core's 16-element batch slice; one
            # fused relu per layer (alternating engines) to minimize hops
            pt7 = psp.tile([128, 4, 2, BT], f32, tag="pt", name="pt7")
            for p in range(4):
                mms(7, p, wsb, p * 1024, pt7[:, p, :, :], X7g=X7g)
            relu_one(0, X[8][:], pt7[:], 1.0 / 16.0)

            pt8 = psp.tile([128, 2, 2, BT], f32, tag="pt", name="pt8")
            for p in range(2):
                mms(8, p, wsb, 4 * 1024 + p * 1024, pt8[:, p, :, :])
            relu_one(1, X[9][:], pt8[:], 1.0 / 16.0)

            pt9 = psp.tile([128, 1, 2, BT], f32, tag="pt", name="pt9")
            mms(9, 0, wsb, 6 * 1024, pt9[:, 0, :, :])
            relu_one(0, X[10][:], pt9[:], 1.0 / 16.0)

            # ---- head: out[b, j] = sum_c X10[c, b] * beta[c, j] / 256
            # (batch-sharded: only this core's 16-element slice)
            ph = php.tile([BT, OUT], f32, tag="ph", name="ph")
            for ch in range(2):
                nc.tensor.matmul(
                    ph[:],
                    X[10][:, 0, ch, :],
                    wsb[:, 7 * 1024 + ch * OUT : 7 * 1024 + (ch + 1) * OUT],
                    start=(ch == 0),
                    stop=(ch == 1),
                )
            ob = sb.tile([BT, OUT], f32, tag="ob", name="ob")
            nc.scalar.activation(ob[:], ph[:], Copy, scale=1.0 / 256.0)
            nc.sync.dma_start(out_d[:], ob[:])

    nc.compile()
    return nc


def _get_nc():
    global _NC
    if _NC is None:
        _NC = _build()
    return _NC


def _prep(inputs):
    x = np.asarray(inputs["x"], dtype=np.float32)
    beta = np.asarray(inputs["beta"], dtype=np.float32)
    ws = [np.asarray(inputs[f"w{l}"], dtype=np.float32) for l in range(10)]

    # x (B,3,1024) -> (kk=2, c=3, d=512, b) fp16
    xk = x.reshape(B, 3, 512, 2).transpose(3, 1, 2, 0).astype(np.float16)
    # w0 (256,3,512,2) -> (kk, c, d, o) fp16
    w0t = ws[0].transpose(3, 1, 2, 0).astype(np.float16)

    # wl (o,c,dl,k) -> [c'=128, (d, kk, ch, oh, o')] fp16
    slabs = {}
    for l in range(1, 10):
        w = ws[l]
        dl = w.shape[2]
        wt = w.reshape(2, 128, 2, 128, dl, 2).transpose(3, 4, 5, 2, 0, 1)
        slabs[l] = (
            np.ascontiguousarray(wt).astype(np.float16).reshape(128, dl * 1024)
        )

    # beta (256,10) -> [c'=128, (ch=2, 10)] fp16
    betat = (
        beta.reshape(2, 128, OUT).transpose(1, 0, 2).astype(np.float16)
    ).reshape(128, 2 * OUT)

    # layers 7-9 weights + beta: replicated on every core (streamed under
    # the collective; the x6 exchange means every core runs layer 7 fully
    # for its batch slice)
    wb = np.ascontiguousarray(
        np.concatenate([slabs[7], slabs[8], slabs[9], betat], axis=1)
    )

    in_maps = []
    for i in range(N_CORES):
        xi = np.ascontiguousarray(xk[:, :, i * 64 : (i + 1) * 64, :]).reshape(
            6, 64 * B
        )
        wi = np.ascontiguousarray(w0t[:, :, i * 64 : (i + 1) * 64, :]).reshape(
            6, 64 * H
        )
        m = {
            "xw0": np.ascontiguousarray(np.concatenate([xi, wi], axis=1)),
            "wa": np.ascontiguousarray(
                np.concatenate(
                    [
                        slabs[l][
                            :,
                            (i * NPOS[l] + p) * 1024 : (i * NPOS[l] + p + 1) * 1024,
                        ]
                        for l, p in POS_A
                    ],
                    axis=1,
                )
            ),
            "wb": wb,
        }
        in_maps.append(m)
    return in_maps


def _run(in_maps, trace=False):
    from concourse import bass_utils

    return bass_utils.run_bass_kernel_spmd(
        _get_nc(), in_maps, core_ids=list(range(N_CORES)), trace=trace
    )


def kernel(**inputs):
    res = _run(_prep(inputs))
    # batch-sharded tail: core c produced output rows [16c, 16c+16)
    return np.concatenate(
        [np.asarray(res.results[c]["out"], dtype=np.float32) for c in range(N_CORES)],
        axis=0,
    )



# revision 51
# speedup vs baseline: 1.0028x; 1.0028x over previous
"""LocallyHierarchicalNet Trainium2 kernel (fp16, weight-stationary,
coalesced weight stream, AllToAll batch-resharded tail).

Net: 10 locally-connected conv1d layers (kernel=stride=2, unshared weights
per position), B=128, C_in=3, H=256, D=1024, then mean + linear head.

Strategy (8 NeuronCores, SPMD):
  - Position-shard layers 0-6: core i owns output positions [i*64,(i+1)*64)
    of layer 0, narrowing to exactly 1 position at layer 6 with zero
    cross-core traffic (binary-tree locality).
  - One 64KB AllToAll reshards from position- to batch-sharding at the
    x6 boundary: core c sends batch slice [16j,16j+16) of its (already
    relu'd fp16) layer-6 output to core j, receiving all 8 x6 positions
    for its own 16-element batch slice. Layers 7-9 + head then run
    batch-sharded (free dim 16), and each core writes output rows
    [16c,16c+16); the host concatenates the 8 slices. The exchanged
    payload is the same either way, but costing the collective on 64KB
    instead of an AllGather's 512KB output saves ~11.5us, and exchanging
    x6 itself (not w7 partials) drops the partial-matmul + cast hop from
    the pre-collective critical path.
  - All weights/activations are fp16 on device (host pre-casts), halving
    the HBM weight stream (the bottleneck: ~271MB of once-used weights,
    ~17MB/core; fp8 was measured at 3.3% final error -- over the gate).
    PSUM accumulation stays f32, so only input rounding (~1e-3) is
    incurred.
  - Matmul layout: weight-stationary. Per output position and O-half:
    psum[o',b] += sum_{kk,ch} W[c',o'] . X[c', b]; lhsT = weight chunk
    [c'=128, o'=128], rhs = activation chunk [c'=128, B] (1 cyc/row fp16).
    Output lands directly as [O, B] = next layer's [C, B]: no transposes.
  - Fused relu+scale+fp16-cast psum->SBUF ops alternate between the
    Activation and DVE engines (GPSIMD cannot read PSUM). L1/L2 bulk and
    L3 positions are processed two per psum tile / relu op; L4 positions
    split per O-half across engines; the L5/L6 root chain uses whole-
    position relus on alternating pinned engines.
  - HBM layout: x0+w0 fused into one tensor; layer-1..6 weights fused into
    one position-major stream in layer order (pooled 2MB slabs for the
    L1/L2 bulk; small dependency-free DMAs into a persistent tile for the
    L3..L6 tail, per-position singles pacing the root chain). Layer-0
    pairs weave between L1 pairs to fill PE while slabs are in flight.
    Layer-7..9 weights + head beta ride in one transfer under the
    collective, held off the pre-collective DMA window by a
    tile_wait_until scheduling floor (the Tile scheduler otherwise hoists
    the dep-free 5us transfer in front of the latency-critical x6 store).
    A tile_wait_until-gated stream of free-dim-512 matmuls keeps the PE
    p-state ramped across the collective so tail matmuls run at full
    clock.

Timeline (cost-model sim, 79.7us total): stream 2-48.7us (DMA-saturated,
360GB/s), root chain + x6 store -> collective launch 53.8us, AllToAll
16.6us (15us constant + 64KB/40GBps), batch-sharded tail + output 9.3us.
The x6 tile is kept chunk-major [p, j, oh, b] so both exchange DMAs run
64B descriptor elements (1024 descriptors, 0.45us) instead of 32B.
"""

import sys

sys.path.insert(0, "/opt/trn_rl_repo")

import numpy as np

N_CORES = 8
B = 128
BT = B // N_CORES  # per-core batch slice for the post-exchange tail layers
C_IN = 3
H = 256
OUT = 10

# per-core output positions per layer (layers 7-9 replicated on all cores)
NPOS = {1: 32, 2: 16, 3: 8, 4: 4, 5: 2, 6: 1, 7: 4, 8: 2, 9: 1}
# position order for the fused layer-1..6 weight stream: plain layer order.
# All L3 weights land ~5us before stream end (absorbing their relu latency),
# and per-position singles for L4/L5/L6 release the root chain step by step.
TAIL = [
    (3, 0), (3, 1), (3, 2), (3, 3), (3, 4), (3, 5), (3, 6), (3, 7),
    (4, 0), (4, 1), (4, 2), (4, 3), (5, 0), (5, 1), (6, 0),
]
POS_A = (
    [(1, p) for p in range(32)] + [(2, p) for p in range(16)] + TAIL
)
# post-exchange layers 7-9 read weights from the replicated wb stream
# slab boundaries: big pooled slabs for the L1/L2 bulk; the interleaved tree
# tail (g 48..62) lives in one persistent SBUF tile filled by small
# dependency-free DMAs so completion sems release compute position-by-position
SLABS_BULK = [(0, 8), (8, 16), (16, 24), (24, 32), (32, 40), (40, 48)]
SLABS_TAIL = [(48, 52), (52, 56), (56, 57), (57, 58), (58, 59), (59, 60),
              (60, 61), (61, 62), (62, 63)]

# scheduler-time floor (ms) for the replicated layer-7..9 weight transfer:
# late enough to keep it off the pre-collective DMA window, early enough to
# finish under the collective
WSB_WAIT_MS = 0.086
KEEPER_WAIT_MS = 0.085

_NC = None


def _build():
    import concourse.bacc as bacc
    import concourse.mybir as mybir
    import concourse.tile as tile

    dt = mybir.dt
    f16 = dt.float16
    f32 = dt.float32
    Relu = mybir.ActivationFunctionType.Relu
    Copy = mybir.ActivationFunctionType.Copy
    Mult = mybir.AluOpType.mult
    Max = mybir.AluOpType.max

    nc = bacc.Bacc(
        "TRN2", target_bir_lowering=False, debug=False, num_devices=N_CORES
    )

    xw0_d = nc.dram_tensor("xw0", [6, 64 * B + 64 * H], f16, kind="ExternalInput")
    wa_d = nc.dram_tensor("wa", [128, 63 * 1024], f16, kind="ExternalInput")
    wb_d = nc.dram_tensor("wb", [128, 7 * 1024 + 2 * OUT], f16, kind="ExternalInput")
    out_d = nc.dram_tensor("out", [BT, OUT], f32, kind="ExternalOutput")

    # relu engine round-robin counter
    rr = [0]

    with tile.TileContext(nc) as tc:
        with (
            tc.tile_pool(name="sb", bufs=1) as sb,
            tc.tile_pool(name="wp", bufs=3) as wp,
            tc.tile_pool(name="psp", bufs=6, space="PSUM") as psp,
            tc.tile_pool(name="php", bufs=1, space="PSUM") as php,
            tc.tile_pool(name="psk", bufs=1, space="PSUM") as psk,
            tc.tile_pool(name="dram", bufs=1, space="DRAM") as dp,
        ):

            # NOTE: GPSIMD/Pool cannot read PSUM (BIR verifier), so the
            # relu/scale/cast work alternates between Activation and DVE only
            def relu_one(e, dst, src, scale):
                if e == 0:
                    nc.scalar.activation(dst, src, Relu, scale=scale)
                else:
                    nc.vector.tensor_scalar(dst, src, scale, 0.0, Mult, Max)

            def relu_store(dst, src, scale):
                relu_one(rr[0] % 2, dst, src, scale)
                rr[0] += 1

            def relu_store2(dst, src, scale):
                # latency-critical: the two O-halves on two different engines
                # so downstream consumers start after half 0
                e = rr[0]
                rr[0] += 1
                relu_one(e % 2, dst[:, 0, :], src[:, 0, :], scale)
                relu_one((e + 1) % 2, dst[:, 1, :], src[:, 1, :], scale)

            xw0 = sb.tile([6, 64 * B + 64 * H], f16, tag="xw0", name="xw0_sb")
            nc.sync.dma_start(xw0[:], xw0_d[:])
            W0 = 64 * B  # offset of the w0 block within xw0

            # activation chain tiles: X[l] is the input of layer l (l=1..10);
            # X[7] is replaced by the exchanged X7g below. Layers 8+ run
            # batch-sharded: only this core's 16-element batch slice.
            X = {1: sb.tile([128, 64, 2, B], f16, tag="x1", name="X1")}
            for l in range(1, 10):
                if l == 6:
                    continue  # x6 lives in the chunk-major X7t tile below
                bl = B if l < 7 else BT
                X[l + 1] = sb.tile(
                    [128, NPOS[l], 2, bl], f16, tag=f"x{l + 1}", name=f"X{l + 1}"
                )
            # x6 lives chunk-major [p, j, oh, b] so the AllToAll store's
            # descriptor runs are 64B (oh,b contiguous on both sides) --
            # half the descriptors of the oh-major layout
            X7t = sb.tile([128, N_CORES, 2, BT], f16, tag="x7", name="X7t")

            # ---- layer 0: K=6 (kk,c_in); lhsT = w0 [6, o-chunk], rhs = x [6, B]
            # output written directly in chain layout [o', oh, B]; two
            # positions share a psum tile and one relu op. L0 pairs are
            # interleaved into the layer-1 slab stream: they depend only on
            # xw0, so they fill PE while weight slabs are still in flight.
            s3 = 1.0 / (3.0**0.5)
            l0_next = [0]

            def l0_pair():
                pos = l0_next[0]
                l0_next[0] += 2
                pt = psp.tile([128, 2, 2, B], f32, tag="pt", name=f"p0_{pos}")
                for q in range(2):
                    for j in range(2):
                        o0 = W0 + (pos + q) * H + j * 128
                        nc.tensor.matmul(
                            pt[:, q, j, :],
                            xw0[:, o0 : o0 + 128],
                            xw0[:, (pos + q) * B : (pos + q + 1) * B],
                            start=True,
                            stop=True,
                        )
                relu_store(X[1][:, pos : pos + 2, :, :], pt[:], s3)

            # front-load enough L0 pairs to keep PE busy until slab 1 lands
            for _ in range(8):
                l0_pair()

            def mms(l, p, ws, off, pslice, X7g=None):
                """The 8 matmuls of one layer-l output position into pslice."""
                Xin = X7g if l == 7 else X[l]
                for oh in range(2):
                    for ci in range(4):
                        kk, ch = ci >> 1, ci & 1
                        o = off + ((kk * 2 + ch) * 2 + oh) * 128
                        nc.tensor.matmul(
                            pslice[:, oh, :],
                            ws[:, o : o + 128],
                            Xin[:, 2 * p + kk, ch, :],
                            start=(ci == 0),
                            stop=(ci == 3),
                        )

            def do_pair(l, p, ws, off):
                """Two consecutive positions, one psum tile, one relu op."""
                pt = psp.tile([128, 2, 2, B], f32, tag="pt", name=f"pt{l}_{p}")
                mms(l, p, ws, off, pt[:, 0, :, :])
                mms(l, p + 1, ws, off + 1024, pt[:, 1, :, :])
                relu_store(X[l + 1][:, p : p + 2, :, :], pt[:], 1.0 / 16.0)

            def do_pos(l, p, ws, off, X7g=None, split=False, eng=None):
                """One position; split=True spreads the relu halves over two
                engines (latency-critical tree tail); eng pins the engine."""
                bl = B if l < 7 else BT
                pt = psp.tile([128, 2, 2, bl], f32, tag="pt", name=f"pt{l}_{p}")
                mms(l, p, ws, off, pt[:, 0, :, :], X7g=X7g)
                if l == 6:
                    # write x6 into the chunk-major tile (strided dst view)
                    dst = X7t.rearrange("p j oh b -> p oh j b")
                    src = pt.rearrange(
                        "p z oh (j b) -> p z oh j b", j=N_CORES
                    )[:, 0]
                    relu_one(eng, dst, src, 1.0 / 16.0)
                    return
                if eng is not None:
                    relu_one(eng, X[l + 1][:, p, :, :], pt[:, 0, :, :], 1.0 / 16.0)
                else:
                    st = relu_store2 if split else relu_store
                    st(X[l + 1][:, p, :, :], pt[:, 0, :, :], 1.0 / 16.0)

            # ---- layers 1-2 bulk: pooled slab stream, positions in pairs,
            # remaining L0 pairs woven between L1 pairs
            for g0, g1 in SLABS_BULK:
                ws = wp.tile([128, (g1 - g0) * 1024], f16, tag="ws", name=f"wsa_{g0}")
                nc.sync.dma_start(ws[:], wa_d[:, g0 * 1024 : g1 * 1024])
                for g in range(g0, g1, 2):
                    l, p = POS_A[g]
                    if l == 1:
                        # keep L0 coverage ahead of this pair's inputs
                        while l0_next[0] < min(64, 2 * p + 12):
                            l0_pair()
                    do_pair(l, p, ws, (g - g0) * 1024)
            while l0_next[0] < 64:
                l0_pair()

            # ---- layer 3-6 tree tail: persistent tile, fine-grained DMAs
            T0 = 48
            wt = sb.tile([128, 15 * 1024], f16, tag="wt", name="wtail")
            for g0, g1 in SLABS_TAIL:
                nc.sync.dma_start(
                    wt[:, (g0 - T0) * 1024 : (g1 - T0) * 1024],
                    wa_d[:, g0 * 1024 : g1 * 1024],
                )
            # L3 (g 48-55) processes positions in pairs: one relu op per two
            # positions halves the fixed per-op engine overhead, relieving
            # ACT/DVE throughput pressure in the window where L4's
            # latency-critical relus also need those engines. Final
            # root-chain positions (g>=56) stay per-position: L4 splits
            # halves across engines, L5/L6 use whole-position relus on
            # alternating pinned engines.
            PIN = {59: 1, 60: 0, 61: 1, 62: 1}
            for g in range(48, 56, 2):
                l, p = POS_A[g]
                do_pair(l, p, wt, (g - T0) * 1024)
            for g in range(56, 63):
                l, p = POS_A[g]
                do_pos(l, p, wt, (g - T0) * 1024, split=(g not in PIN),
                       eng=PIN.get(g))

            # PE warm-keeper: dependency-free free-dim-512 matmuls in a
            # dedicated psum bank keep the PE p-state ramped across the
            # collective's idle gap (~20us) so the post-exchange layer
            # matmuls run at full clock the moment their data lands.
            wpt = psk.tile([128, 512], f32, tag="wpt", name="warm_pt")
            with tc.tile_wait_until(KEEPER_WAIT_MS):
                for _ in range(110):
                    nc.tensor.matmul(
                        wpt[:],
                        xw0[:, 0:128],
                        xw0[:, B : B + 512],
                        start=True,
                        stop=True,
                    )

            # ---- AllToAll the layer-6 outputs (x6, already relu'd fp16 in
            # X[7], so no extra cast/partial hop before the exchange): chunk
            # j of this core's x6 (batch slice [16j,16j+16)) goes to core j;
            # output chunk j is core j's x6 position for THIS core's batch
            # slice. 64KB each way.
            ag_in = dp.tile([N_CORES, 128, 2, BT], f16, name="ag_in")
            ag_out = dp.tile([N_CORES, 128, 2, BT], f16, name="ag_out")
            nc.sync.dma_start(
                ag_in.rearrange("j p oh b -> p j oh b"), X7t[:]
            )
            nc.gpsimd.collective_compute(
                "AllToAll",
                mybir.AluOpType.bypass,
                replica_groups=[list(range(N_CORES))],
                ins=[ag_in.opt()],
                outs=[ag_out.opt()],
            )

            # layers 7-9 weights + beta: single transfer, hidden under the
            # collective. tile_wait_until keeps the Tile scheduler from
            # hoisting this dep-free 5us transfer into the DMA window the
            # latency-critical ag store needs just before the collective.
            wsb = wp.tile([128, 7 * 1024 + 2 * OUT], f16, tag="ws", name="wsb")
            with tc.tile_wait_until(WSB_WAIT_MS):
                nc.sync.dma_start(wsb[:], wb_d[:])

            # exchanged x6 (all 8 positions) for this core's batch slice
            X7g = sb.tile([128, 8, 2, BT], f16, tag="x7g", name="X7g")
            nc.sync.dma_start(X7g[:], ag_out.rearrange("j p oh b -> p j oh b"))

            # ---- layers 7-9 for this core's 16-element batch slice; one
            # fused relu per layer (alternating engines) to minimize hops
            pt7 = psp.tile([128, 4, 2, BT], f32, tag="pt", name="pt7")
            for p in range(4):
                mms(7, p, wsb, p * 1024, pt7[:, p, :, :], X7g=X7g)
            relu_one(1, X[8][:], pt7[:], 1.0 / 16.0)

            pt8 = psp.tile([128, 2, 2, BT], f32, tag="pt", name="pt8")
            for p in range(2):
                mms(8, p, wsb, 4 * 1024 + p * 1024, pt8[:, p, :, :])
            relu_one(1, X[9][:], pt8[:], 1.0 / 16.0)

            pt9 = psp.tile([128, 1, 2, BT], f32, tag="pt", name="pt9")
            mms(9, 0, wsb, 6 * 1024, pt9[:, 0, :, :])
            relu_one(1, X[10][:], pt9[:], 1.0 / 16.0)

            # ---- head: out[b, j] = sum_c X10[c, b] * beta[c, j] / 256
            # (batch-sharded: only this core's 16-element slice)
            ph = php.tile([BT, OUT], f32, tag="ph", name="ph")
            for ch in range(2):
                nc.tensor.matmul(
                    ph[:],
                    X[10][:, 0, ch, :],
                    wsb[:, 7 * 1024 + ch * OUT : 7 * 1024 + (ch + 1) * OUT],
                    start=(ch == 0),
                    stop=(ch == 1),
                )
            ob = sb.tile([BT, OUT], f32, tag="ob", name="ob")
            nc.vector.tensor_scalar_mul(ob[:], ph[:], 1.0 / 256.0)
            nc.sync.dma_start(out_d[:], ob[:])

    nc.compile()
    return nc


def _get_nc():
    global _NC
    if _NC is None:
        _NC = _build()
    return _NC


def _prep(inputs):
    x = np.asarray(inputs["x"], dtype=np.float32)
    beta = np.asarray(inputs["beta"], dtype=np.float32)
    ws = [np.asarray(inputs[f"w{l}"], dtype=np.float32) for l in range(10)]

    # x (B,3,1024) -> (kk=2, c=3, d=512, b) fp16
    xk = x.reshape(B, 3, 512, 2).transpose(3, 1, 2, 0).astype(np.float16)
    # w0 (256,3,512,2) -> (kk, c, d, o) fp16
    w0t = ws[0].transpose(3, 1, 2, 0).astype(np.float16)

    # wl (o,c,dl,k) -> [c'=128, (d, kk, ch, oh, o')] fp16
    slabs = {}
    for l in range(1, 10):
        w = ws[l]
        dl = w.shape[2]
        wt = w.reshape(2, 128, 2, 128, dl, 2).transpose(3, 4, 5, 2, 0, 1)
        slabs[l] = (
            np.ascontiguousarray(wt).astype(np.float16).reshape(128, dl * 1024)
        )

    # beta (256,10) -> [c'=128, (ch=2, 10)] fp16
    betat = (
        beta.reshape(2, 128, OUT).transpose(1, 0, 2).astype(np.float16)
    ).reshape(128, 2 * OUT)

    # layers 7-9 weights + beta: replicated on every core (streamed under
    # the collective; the x6 exchange means every core runs layer 7 fully
    # for its batch slice)
    wb = np.ascontiguousarray(
        np.concatenate([slabs[7], slabs[8], slabs[9], betat], axis=1)
    )

    in_maps = []
    for i in range(N_CORES):
        xi = np.ascontiguousarray(xk[:, :, i * 64 : (i + 1) * 64, :]).reshape(
            6, 64 * B
        )
        wi = np.ascontiguousarray(w0t[:, :, i * 64 : (i + 1) * 64, :]).reshape(
            6, 64 * H
        )
        m = {
            "xw0": np.ascontiguousarray(np.concatenate([xi, wi], axis=1)),
            "wa": np.ascontiguousarray(
                np.concatenate(
                    [
                        slabs[l][
                            :,
                            (i * NPOS[l] + p) * 1024 : (i * NPOS[l] + p + 1) * 1024,
                        ]
                        for l, p in POS_A
                    ],
                    axis=1,
                )
            ),
            "wb": wb,
        }
        in_maps.append(m)
    return in_maps


def _run(in_maps, trace=False):
    from concourse import bass_utils

    return bass_utils.run_bass_kernel_spmd(
        _get_nc(), in_maps, core_ids=list(range(N_CORES)), trace=trace
    )


def kernel(**inputs):
    res = _run(_prep(inputs))
    # batch-sharded tail: core c produced output rows [16c, 16c+16)
    return np.concatenate(
        [np.asarray(res.results[c]["out"], dtype=np.float32) for c in range(N_CORES)],
        axis=0,
    )



# revision 52
# speedup vs baseline: 1.0139x; 1.0110x over previous
"""LocallyHierarchicalNet Trainium2 kernel (fp16, weight-stationary,
coalesced weight stream, AllToAll batch-resharded tail).

Net: 10 locally-connected conv1d layers (kernel=stride=2, unshared weights
per position), B=128, C_in=3, H=256, D=1024, then mean + linear head.

Strategy (8 NeuronCores, SPMD):
  - Position-shard layers 0-6: core i owns output positions [i*64,(i+1)*64)
    of layer 0, narrowing to exactly 1 position at layer 6 with zero
    cross-core traffic (binary-tree locality).
  - One 64KB AllToAll reshards from position- to batch-sharding at the
    x6 boundary: core c sends batch slice [16j,16j+16) of its (already
    relu'd fp16) layer-6 output to core j, receiving all 8 x6 positions
    for its own 16-element batch slice. Layers 7-9 + head then run
    batch-sharded (free dim 16), and each core writes output rows
    [16c,16c+16); the host concatenates the 8 slices. The exchanged
    payload is the same either way, but costing the collective on 64KB
    instead of an AllGather's 512KB output saves ~11.5us, and exchanging
    x6 itself (not w7 partials) drops the partial-matmul + cast hop from
    the pre-collective critical path.
  - All weights/activations are fp16 on device (host pre-casts), halving
    the HBM weight stream (the bottleneck: ~271MB of once-used weights,
    ~17MB/core; fp8 was measured at 3.3% final error -- over the gate).
    PSUM accumulation stays f32, so only input rounding (~1e-3) is
    incurred.
  - Matmul layout: weight-stationary. Per output position and O-half:
    psum[o',b] += sum_{kk,ch} W[c',o'] . X[c', b]; lhsT = weight chunk
    [c'=128, o'=128], rhs = activation chunk [c'=128, B] (1 cyc/row fp16).
    Output lands directly as [O, B] = next layer's [C, B]: no transposes.
  - Fused relu+scale+fp16-cast psum->SBUF ops alternate between the
    Activation and DVE engines (GPSIMD cannot read PSUM). L1/L2 bulk and
    L3 positions are processed two per psum tile / relu op; L4 positions
    split per O-half across engines; the L5/L6 root chain uses whole-
    position relus on alternating pinned engines.
  - HBM layout: x0+w0 fused into one tensor; layer-1..6 weights fused into
    one position-major stream in layer order (pooled 2MB slabs for the
    L1/L2 bulk; small dependency-free DMAs into a persistent tile for the
    L3..L6 tail, per-position singles pacing the root chain). Layer-0
    pairs weave between L1 pairs to fill PE while slabs are in flight.
    Layer-7..9 weights + head beta ride in one transfer under the
    collective, held off the pre-collective DMA window by a
    tile_wait_until scheduling floor (the Tile scheduler otherwise hoists
    the dep-free 5us transfer in front of the latency-critical x6 store).
    A tile_wait_until-gated stream of free-dim-512 matmuls keeps the PE
    p-state ramped across the collective so tail matmuls run at full
    clock.

Timeline (cost-model sim, 79.3us total): stream 2-48.7us (DMA-saturated,
360GB/s), root chain + x6 store -> collective launch 53.7us, AllToAll
16.6us (15us constant + 64KB/40GBps), batch-sharded tail + output 8.9us.
The x6 tile is kept chunk-major [p, j, oh, b] so both exchange DMAs run
64B descriptor elements (1024 descriptors, 0.45us) instead of 32B. The
latency-critical L5p1/L6 chain relus and all post-exchange relus/copies
run on the DVE engine, whose fixed per-op overhead is ~100ns lower than
Activation's at these small (<=256 elem/partition) psum->SBUF sizes.
"""

import sys

sys.path.insert(0, "/opt/trn_rl_repo")

import numpy as np

N_CORES = 8
B = 128
BT = B // N_CORES  # per-core batch slice for the post-exchange tail layers
C_IN = 3
H = 256
OUT = 10

# per-core output positions per layer (layers 7-9 replicated on all cores)
NPOS = {1: 32, 2: 16, 3: 8, 4: 4, 5: 2, 6: 1, 7: 4, 8: 2, 9: 1}
# position order for the fused layer-1..6 weight stream: plain layer order.
# All L3 weights land ~5us before stream end (absorbing their relu latency),
# and per-position singles for L4/L5/L6 release the root chain step by step.
TAIL = [
    (3, 0), (3, 1), (3, 2), (3, 3), (3, 4), (3, 5), (3, 6), (3, 7),
    (4, 0), (4, 1), (4, 2), (4, 3), (5, 0), (5, 1), (6, 0),
]
POS_A = (
    [(1, p) for p in range(32)] + [(2, p) for p in range(16)] + TAIL
)
# post-exchange layers 7-9 read weights from the replicated wb stream
# slab boundaries: big pooled slabs for the L1/L2 bulk; the interleaved tree
# tail (g 48..62) lives in one persistent SBUF tile filled by small
# dependency-free DMAs so completion sems release compute position-by-position
SLABS_BULK = [(0, 8), (8, 16), (16, 24), (24, 32), (32, 40), (40, 48)]
SLABS_TAIL = [(48, 52), (52, 56), (56, 57), (57, 58), (58, 59), (59, 60),
              (60, 61), (61, 62), (62, 63)]

# scheduler-time floor (ms) for the replicated layer-7..9 weight transfer:
# late enough to keep it off the pre-collective DMA window, early enough to
# finish under the collective
WSB_WAIT_MS = 0.086
KEEPER_WAIT_MS = 0.085

_NC = None


def _build():
    import concourse.bacc as bacc
    import concourse.mybir as mybir
    import concourse.tile as tile

    dt = mybir.dt
    f16 = dt.float16
    f32 = dt.float32
    Relu = mybir.ActivationFunctionType.Relu
    Copy = mybir.ActivationFunctionType.Copy
    Mult = mybir.AluOpType.mult
    Max = mybir.AluOpType.max

    nc = bacc.Bacc(
        "TRN2", target_bir_lowering=False, debug=False, num_devices=N_CORES
    )

    xw0_d = nc.dram_tensor("xw0", [6, 64 * B + 64 * H], f16, kind="ExternalInput")
    wa_d = nc.dram_tensor("wa", [128, 63 * 1024], f16, kind="ExternalInput")
    wb_d = nc.dram_tensor("wb", [128, 7 * 1024 + 2 * OUT], f16, kind="ExternalInput")
    out_d = nc.dram_tensor("out", [BT, OUT], f32, kind="ExternalOutput")

    # relu engine round-robin counter
    rr = [0]

    with tile.TileContext(nc) as tc:
        with (
            tc.tile_pool(name="sb", bufs=1) as sb,
            tc.tile_pool(name="wp", bufs=3) as wp,
            tc.tile_pool(name="psp", bufs=6, space="PSUM") as psp,
            tc.tile_pool(name="php", bufs=1, space="PSUM") as php,
            tc.tile_pool(name="psk", bufs=1, space="PSUM") as psk,
            tc.tile_pool(name="dram", bufs=1, space="DRAM") as dp,
        ):

            # NOTE: GPSIMD/Pool cannot read PSUM (BIR verifier), so the
            # relu/scale/cast work alternates between Activation and DVE only
            def relu_one(e, dst, src, scale):
                if e == 0:
                    nc.scalar.activation(dst, src, Relu, scale=scale)
                else:
                    nc.vector.tensor_scalar(dst, src, scale, 0.0, Mult, Max)

            def relu_store(dst, src, scale):
                relu_one(rr[0] % 2, dst, src, scale)
                rr[0] += 1

            def relu_store2(dst, src, scale):
                # latency-critical: the two O-halves on two different engines
                # so downstream consumers start after half 0
                e = rr[0]
                rr[0] += 1
                relu_one(e % 2, dst[:, 0, :], src[:, 0, :], scale)
                relu_one((e + 1) % 2, dst[:, 1, :], src[:, 1, :], scale)

            xw0 = sb.tile([6, 64 * B + 64 * H], f16, tag="xw0", name="xw0_sb")
            nc.sync.dma_start(xw0[:], xw0_d[:])
            W0 = 64 * B  # offset of the w0 block within xw0

            # activation chain tiles: X[l] is the input of layer l (l=1..10);
            # X[7] is replaced by the exchanged X7g below. Layers 8+ run
            # batch-sharded: only this core's 16-element batch slice.
            X = {1: sb.tile([128, 64, 2, B], f16, tag="x1", name="X1")}
            for l in range(1, 10):
                if l == 6:
                    continue  # x6 lives in the chunk-major X7t tile below
                bl = B if l < 7 else BT
                X[l + 1] = sb.tile(
                    [128, NPOS[l], 2, bl], f16, tag=f"x{l + 1}", name=f"X{l + 1}"
                )
            # x6 lives chunk-major [p, j, oh, b] so the AllToAll store's
            # descriptor runs are 64B (oh,b contiguous on both sides) --
            # half the descriptors of the oh-major layout
            X7t = sb.tile([128, N_CORES, 2, BT], f16, tag="x7", name="X7t")

            # ---- layer 0: K=6 (kk,c_in); lhsT = w0 [6, o-chunk], rhs = x [6, B]
            # output written directly in chain layout [o', oh, B]; two
            # positions share a psum tile and one relu op. L0 pairs are
            # interleaved into the layer-1 slab stream: they depend only on
            # xw0, so they fill PE while weight slabs are still in flight.
            s3 = 1.0 / (3.0**0.5)
            l0_next = [0]

            def l0_pair():
                pos = l0_next[0]
                l0_next[0] += 2
                pt = psp.tile([128, 2, 2, B], f32, tag="pt", name=f"p0_{pos}")
                for q in range(2):
                    for j in range(2):
                        o0 = W0 + (pos + q) * H + j * 128
                        nc.tensor.matmul(
                            pt[:, q, j, :],
                            xw0[:, o0 : o0 + 128],
                            xw0[:, (pos + q) * B : (pos + q + 1) * B],
                            start=True,
                            stop=True,
                        )
                relu_store(X[1][:, pos : pos + 2, :, :], pt[:], s3)

            # front-load enough L0 pairs to keep PE busy until slab 1 lands
            for _ in range(8):
                l0_pair()

            def mms(l, p, ws, off, pslice, X7g=None):
                """The 8 matmuls of one layer-l output position into pslice."""
                Xin = X7g if l == 7 else X[l]
                for oh in range(2):
                    for ci in range(4):
                        kk, ch = ci >> 1, ci & 1
                        o = off + ((kk * 2 + ch) * 2 + oh) * 128
                        nc.tensor.matmul(
                            pslice[:, oh, :],
                            ws[:, o : o + 128],
                            Xin[:, 2 * p + kk, ch, :],
                            start=(ci == 0),
                            stop=(ci == 3),
                        )

            def do_pair(l, p, ws, off):
                """Two consecutive positions, one psum tile, one relu op."""
                pt = psp.tile([128, 2, 2, B], f32, tag="pt", name=f"pt{l}_{p}")
                mms(l, p, ws, off, pt[:, 0, :, :])
                mms(l, p + 1, ws, off + 1024, pt[:, 1, :, :])
                relu_store(X[l + 1][:, p : p + 2, :, :], pt[:], 1.0 / 16.0)

            def do_pos(l, p, ws, off, X7g=None, split=False, eng=None):
                """One position; split=True spreads the relu halves over two
                engines (latency-critical tree tail); eng pins the engine."""
                bl = B if l < 7 else BT
                pt = psp.tile([128, 2, 2, bl], f32, tag="pt", name=f"pt{l}_{p}")
                mms(l, p, ws, off, pt[:, 0, :, :], X7g=X7g)
                if l == 6:
                    # write x6 into the chunk-major tile (strided dst view)
                    dst = X7t.rearrange("p j oh b -> p oh j b")
                    src = pt.rearrange(
                        "p z oh (j b) -> p z oh j b", j=N_CORES
                    )[:, 0]
                    relu_one(eng, dst, src, 1.0 / 16.0)
                    return
                if eng is not None:
                    relu_one(eng, X[l + 1][:, p, :, :], pt[:, 0, :, :], 1.0 / 16.0)
                else:
                    st = relu_store2 if split else relu_store
                    st(X[l + 1][:, p, :, :], pt[:, 0, :, :], 1.0 / 16.0)

            # ---- layers 1-2 bulk: pooled slab stream, positions in pairs,
            # remaining L0 pairs woven between L1 pairs
            for g0, g1 in SLABS_BULK:
                ws = wp.tile([128, (g1 - g0) * 1024], f16, tag="ws", name=f"wsa_{g0}")
                nc.sync.dma_start(ws[:], wa_d[:, g0 * 1024 : g1 * 1024])
                for g in range(g0, g1, 2):
                    l, p = POS_A[g]
                    if l == 1:
                        # keep L0 coverage ahead of this pair's inputs
                        while l0_next[0] < min(64, 2 * p + 12):
                            l0_pair()
                    do_pair(l, p, ws, (g - g0) * 1024)
            while l0_next[0] < 64:
                l0_pair()

            # ---- layer 3-6 tree tail: persistent tile, fine-grained DMAs
            T0 = 48
            wt = sb.tile([128, 15 * 1024], f16, tag="wt", name="wtail")
            for g0, g1 in SLABS_TAIL:
                nc.sync.dma_start(
                    wt[:, (g0 - T0) * 1024 : (g1 - T0) * 1024],
                    wa_d[:, g0 * 1024 : g1 * 1024],
                )
            # L3 (g 48-55) processes positions in pairs: one relu op per two
            # positions halves the fixed per-op engine overhead, relieving
            # ACT/DVE throughput pressure in the window where L4's
            # latency-critical relus also need those engines. Final
            # root-chain positions (g>=56) stay per-position: L4 splits
            # halves across engines, L5/L6 use whole-position relus on
            # alternating pinned engines.
            PIN = {59: 1, 60: 0, 61: 1, 62: 1}
            for g in range(48, 56, 2):
                l, p = POS_A[g]
                do_pair(l, p, wt, (g - T0) * 1024)
            for g in range(56, 63):
                l, p = POS_A[g]
                do_pos(l, p, wt, (g - T0) * 1024, split=(g not in PIN),
                       eng=PIN.get(g))

            # PE warm-keeper: dependency-free free-dim-512 matmuls in a
            # dedicated psum bank keep the PE p-state ramped across the
            # collective's idle gap (~20us) so the post-exchange layer
            # matmuls run at full clock the moment their data lands.
            wpt = psk.tile([128, 512], f32, tag="wpt", name="warm_pt")
            with tc.tile_wait_until(KEEPER_WAIT_MS):
                for _ in range(110):
                    nc.tensor.matmul(
                        wpt[:],
                        xw0[:, 0:128],
                        xw0[:, B : B + 512],
                        start=True,
                        stop=True,
                    )

            # ---- AllToAll the layer-6 outputs (x6, already relu'd fp16 in
            # X[7], so no extra cast/partial hop before the exchange): chunk
            # j of this core's x6 (batch slice [16j,16j+16)) goes to core j;
            # output chunk j is core j's x6 position for THIS core's batch
            # slice. 64KB each way.
            ag_in = dp.tile([N_CORES, 128, 2, BT], f16, name="ag_in")
            ag_out = dp.tile([N_CORES, 128, 2, BT], f16, name="ag_out")
            nc.sync.dma_start(
                ag_in.rearrange("j p oh b -> p j oh b"), X7t[:]
            )
            nc.gpsimd.collective_compute(
                "AllToAll",
                mybir.AluOpType.bypass,
                replica_groups=[list(range(N_CORES))],
                ins=[ag_in.opt()],
                outs=[ag_out.opt()],
            )

            # layers 7-9 weights + beta: single transfer, hidden under the
            # collective. tile_wait_until keeps the Tile scheduler from
            # hoisting this dep-free 5us transfer into the DMA window the
            # latency-critical ag store needs just before the collective.
            wsb = wp.tile([128, 7 * 1024 + 2 * OUT], f16, tag="ws", name="wsb")
            with tc.tile_wait_until(WSB_WAIT_MS):
                nc.sync.dma_start(wsb[:], wb_d[:])

            # exchanged x6 (all 8 positions) for this core's batch slice
            X7g = sb.tile([128, 8, 2, BT], f16, tag="x7g", name="X7g")
            nc.sync.dma_start(X7g[:], ag_out.rearrange("j p oh b -> p j oh b"))

            # ---- layers 7-9 for this core's 16-element batch slice; one
            # fused relu per layer (alternating engines) to minimize hops
            pt7 = psp.tile([128, 4, 2, BT], f32, tag="pt", name="pt7")
            for p in range(4):
                mms(7, p, wsb, p * 1024, pt7[:, p, :, :], X7g=X7g)
            relu_one(1, X[8][:], pt7[:], 1.0 / 16.0)

            pt8 = psp.tile([128, 2, 2, BT], f32, tag="pt", name="pt8")
            for p in range(2):
                mms(8, p, wsb, 4 * 1024 + p * 1024, pt8[:, p, :, :])
            relu_one(1, X[9][:], pt8[:], 1.0 / 16.0)

            pt9 = psp.tile([128, 1, 2, BT], f32, tag="pt", name="pt9")
            mms(9, 0, wsb, 6 * 1024, pt9[:, 0, :, :])
            relu_one(1, X[10][:], pt9[:], 1.0 / 16.0)

            # ---- head: out[b, j] = sum_c X10[c, b] * beta[c, j] / 256
            # (batch-sharded: only this core's 16-element slice)
            ph = php.tile([BT, OUT], f32, tag="ph", name="ph")
            for ch in range(2):
                nc.tensor.matmul(
                    ph[:],
                    X[10][:, 0, ch, :],
                    wsb[:, 7 * 1024 + ch * OUT : 7 * 1024 + (ch + 1) * OUT],
                    start=(ch == 0),
                    stop=(ch == 1),
                )
            ob = sb.tile([BT, OUT], f32, tag="ob", name="ob")
            nc.vector.tensor_scalar_mul(ob[:], ph[:], 1.0 / 256.0)
            nc.sync.dma_start(out_d[:], ob[:])

    nc.compile()
    return nc


def _get_nc():
    global _NC
    if _NC is None:
        _NC = _build()
    return _NC


def _prep(inputs):
    x = np.asarray(inputs["x"], dtype=np.float32)
    beta = np.asarray(inputs["beta"], dtype=np.float32)
    ws = [np.asarray(inputs[f"w{l}"], dtype=np.float32) for l in range(10)]

    # x (B,3,1024) -> (kk=2, c=3, d=512, b) fp16
    xk = x.reshape(B, 3, 512, 2).transpose(3, 1, 2, 0).astype(np.float16)
    # w0 (256,3,512,2) -> (kk, c, d, o) fp16
    w0t = ws[0].transpose(3, 1, 2, 0).astype(np.float16)

    # wl (o,c,dl,k) -> [c'=128, (d, kk, ch, oh, o')] fp16
    slabs = {}
    for l in range(1, 10):
        w = ws[l]
        dl = w.shape[2]
        wt = w.reshape(2, 128, 2, 128, dl, 2).transpose(3, 4, 5, 2, 0, 1)
        slabs[l] = (
            np.ascontiguousarray(wt).astype(np.float16).reshape(128, dl * 1024)
        )

    # beta (256,10) -> [c'=128, (ch=2, 10)] fp16
    betat = (
        beta.reshape(2, 128, OUT).transpose(1, 0, 2).astype(np.float16)
    ).reshape(128, 2 * OUT)

    # layers 7-9 weights + beta: replicated on every core (streamed under
    # the collective; the x6 exchange means every core runs layer 7 fully
    # for its batch slice)
    wb = np.ascontiguousarray(
        np.concatenate([slabs[7], slabs[8], slabs[9], betat], axis=1)
    )

    in_maps = []
    for i in range(N_CORES):
        xi = np.ascontiguousarray(xk[:, :, i * 64 : (i + 1) * 64, :]).reshape(
            6, 64 * B
        )
        wi = np.ascontiguousarray(w0t[:, :, i * 64 : (i + 1) * 64, :]).reshape(
            6, 64 * H
        )
        m = {
            "xw0": np.ascontiguousarray(np.concatenate([xi, wi], axis=1)),
            "wa": np.ascontiguousarray(
                np.concatenate(
                    [
                        slabs[l][
                            :,
                            (i * NPOS[l] + p) * 1024 : (i * NPOS[l] + p + 1) * 1024,
                        ]
                        for l, p in POS_A
                    ],
                    axis=1,
                )
            ),
            "wb": wb,
        }
        in_maps.append(m)
    return in_maps


def _run(in_maps, trace=False):
    from concourse import bass_utils

    return bass_utils.run_bass_kernel_spmd(
        _get_nc(), in_maps, core_ids=list(range(N_CORES)), trace=trace
    )


def kernel(**inputs):
    res = _run(_prep(inputs))
    # batch-sharded tail: core c produced output rows [16c, 16c+16)
    return np.concatenate(
        [np.asarray(res.results[c]["out"], dtype=np.float32) for c in range(N_CORES)],
        axis=0,
    )

